# revision 15
# baseline (speedup 1.0000x reference)
"""Multi-head attention (B=2, S=2048, D=1024, H=16) on 8 Trainium2 cores.

Sharding: core c = (batch b, head-group hg) with b = c // 4, hg = c % 4.
Each core computes 4 heads of one batch element end-to-end and emits a
partial output projection; the host sums the 4 partials per batch and adds
bo + bv @ Wo (the V-bias commutes through softmax-normalized attention).

Performance-critical structure (v6):
  - ALL device inputs are pre-packed on the host into partition-major
    layouts (x as [NB*128, KD*SBLK] bf16 blocks, weights as [128, kd*n])
    so every DMA is one large contiguous descriptor per partition — the
    descriptor-fragmented rearranges cost a 16 us DMA head and ~30% of
    aggregate bandwidth in earlier revisions.
  - All matmul operands bf16; PSUM fp32. Projection accumulation chains
    interleave across PSUM banks (4-way for the 256-free V chains) so the
    accumulate turnaround never serializes the PE.
  - Phase B is paced by ScalarE exp (16.8M exps/core, ~1.14 us per
    [128,1024] tile = the hard floor). Score chunks are per-j-tile with both
    heads packed in one 2-bank PSUM tile (3 rotate; PV accumulators pin 2),
    exp->PV runs 2 chunks deep through a global chunk stream that crosses
    head-pair/block boundaries, and softmax normalize runs entirely on
    DVE+GPSIMD (fast reciprocal + partition broadcast).
  - q-projections for blocks 1..3 (split per head-pair) and finished
    blocks' out-projections drip into the chunk stream as PE filler.
  - Tail: the last block's out-projection splits by head-pair (t=0 early
    into `out`, t=1 into `out1`, summed host-side) and its normalize runs
    per-head directly from PSUM to shorten the critical chain.
"""

import numpy as np

import concourse.bacc as bacc
import concourse.mybir as mybir
import concourse.tile as tile
from concourse.bass_utils import run_bass_kernel_spmd

F32 = mybir.dt.float32
BF16 = mybir.dt.bfloat16

S_FULL, D_FULL, NH_PER_CORE, DH = 2048, 1024, 4, 64
N_CORES = 8
B_FULL, H_FULL = 2, 16


def build_core_program(S=S_FULL, D=D_FULL, NH=NH_PER_CORE):
    """One core's program. Packed inputs (bf16 unless noted):
      xqt/xkt/xvt [NB*P, KD*SBLK]  x^T blocks, partition-major
      wq/wk/wv    [P, KD*NSL]      projection weights, partition-major
      wo          [P, NT*D]        out-proj weights, partition-major
      bq/bk       [P, NT] f32
    Outputs: out [S,D] (partial; last block rows hold only the t=0 half)
    and out1 [SBLK,D] (last block t=1 half)."""
    NSL = NH * DH
    P = 128
    KD = D // P
    NT = NSL // P
    SBLK = 512 if S % 512 == 0 else S
    NB = S // SBLK
    JT = S // P
    SS = SBLK // P

    nc = bacc.Bacc("TRN2", target_bir_lowering=False, debug=False)

    xq_d = nc.dram_tensor("xqt", [NB * P, KD * SBLK], BF16,
                          kind="ExternalInput")
    xk_d = nc.dram_tensor("xkt", [NB * P, KD * SBLK], BF16,
                          kind="ExternalInput")
    xv_d = nc.dram_tensor("xvt", [NB * P, KD * SBLK], BF16,
                          kind="ExternalInput")
    wq_d = nc.dram_tensor("wq", [P, KD * NSL], BF16, kind="ExternalInput")
    wk_d = nc.dram_tensor("wk", [P, KD * NSL], BF16, kind="ExternalInput")
    wv_d = nc.dram_tensor("wv", [P, KD * NSL], BF16, kind="ExternalInput")
    wo_d = nc.dram_tensor("wo", [P, NT * D], BF16, kind="ExternalInput")
    bq_d = nc.dram_tensor("bq", [P, NT], F32, kind="ExternalInput")
    bk_d = nc.dram_tensor("bk", [P, NT], F32, kind="ExternalInput")
    out_d = nc.dram_tensor("out", [S, D], BF16, kind="ExternalOutput")
    out1_d = nc.dram_tensor("out1", [SBLK, D], BF16, kind="ExternalOutput")

    def xblk(x_d, blk):
        return x_d[blk * P : (blk + 1) * P, :].rearrange(
            "p (kd s) -> p kd s", kd=KD
        )

    with tile.TileContext(nc) as tc:
        with tc.tile_pool(name="persist", bufs=1) as pp:
            wv_sb = pp.tile([P, KD, NSL], BF16)
            nc.sync.dma_start(
                wv_sb, wv_d.rearrange("p (kd n) -> p kd n", kd=KD)
            )
            wk_sb = pp.tile([P, KD, NSL], BF16)
            wq_sb = pp.tile([P, KD, NSL], BF16)
            wo_sb = pp.tile([P, NT, D], BF16)
            bq_sb = pp.tile([P, NT], F32)
            bk_sb = pp.tile([P, NT], F32)

            qT_b = [
                pp.tile([P, NT, SBLK], BF16, name=f"qT{b}") for b in range(NB)
            ]
            kT = pp.tile([P, NT, S], BF16)
            v_sb = pp.tile([P, JT, NH, DH + 1], BF16)
            nc.vector.memset(v_sb[:, :, :, DH : DH + 1], 1.0)
            o_b = [
                pp.tile([P, NT, SBLK], BF16, name=f"o{b}") for b in range(NB)
            ]

            with tc.tile_pool(name="stage", bufs=3) as pa:
                # ---- Phase A: v + k projections for all blocks, q block 0.
                with tc.tile_pool(name="psa", bufs=2, space="PSUM") as psa:
                    # Pre-ramp: the PE DVFS reaches its top p-state only
                    # after a few microseconds of continuous execution, and
                    # the input DMA head would otherwise leave it cold for
                    # the first half of the v-projection. Chew ~10 us of
                    # dependency-free dummy matmuls while the DMA lands.
                    dum = pp.tile([P, 128], BF16)
                    nc.vector.memset(dum, 0.0)
                    psd = [
                        psa.tile([P, 128], F32, tag="psv", bufs=4,
                                 name=f"psd{i}")
                        for i in range(2)
                    ]
                    for i in range(56):
                        nc.tensor.matmul(
                            psd[i % 2],
                            lhsT=dum,
                            rhs=dum,
                            start=True,
                            stop=True,
                        )
                    for kind, x_d in (("v", xv_d), ("k", xk_d), ("q0", xq_d)):
                        blks = range(NB) if kind != "q0" else range(1)
                        for blk in blks:
                            xt = pa.tile([P, KD, SBLK], BF16, tag="xt")
                            nc.sync.dma_start(xt, xblk(x_d, blk))
                            if kind == "v" and blk == 0:
                                nc.sync.dma_start(
                                    wk_sb,
                                    wk_d.rearrange("p (kd n) -> p kd n",
                                                   kd=KD),
                                )
                                nc.sync.dma_start(
                                    wq_sb,
                                    wq_d.rearrange("p (kd n) -> p kd n",
                                                   kd=KD),
                                )
                                nc.sync.dma_start(
                                    wo_sb,
                                    wo_d.rearrange("p (t n) -> p t n", t=NT),
                                )
                                nc.sync.dma_start(bq_sb, bq_d[:, :])
                                nc.sync.dma_start(bk_sb, bk_d[:, :])
                            if kind == "v":
                                # 4-way bank interleave: at full clock a
                                # 256-cycle matmul is shorter than the
                                # accumulate turnaround, so 2-way is not
                                # enough to keep the chains pipelined.
                                psv = [
                                    psa.tile([P, NSL], F32, tag="psv",
                                             bufs=4, name=f"psv{i}")
                                    for i in range(SS)
                                ]
                                for kd in range(KD):
                                    for ss in range(SS):
                                        nc.tensor.matmul(
                                            psv[ss],
                                            lhsT=xt[:, kd,
                                                    ss * P : (ss + 1) * P],
                                            rhs=wv_sb[:, kd, :],
                                            start=(kd == 0),
                                            stop=(kd == KD - 1),
                                        )
                                for ss in range(SS):
                                    st = blk * SS + ss
                                    nc.vector.tensor_copy(
                                        v_sb[:, st, :, 0:DH],
                                        psv[ss].rearrange(
                                            "p (h d) -> p h d", d=DH
                                        ),
                                    )
                            else:
                                w_sb = wk_sb if kind == "k" else wq_sb
                                b_sb = bk_sb if kind == "k" else bq_sb
                                psp = [
                                    psa.tile([P, SBLK], F32, tag="psp",
                                             bufs=4, name=f"psp{i}")
                                    for i in range(NT)
                                ]
                                for kd in range(KD):
                                    for nt in range(NT):
                                        nc.tensor.matmul(
                                            psp[nt],
                                            lhsT=w_sb[:, kd,
                                                      nt * P : (nt + 1) * P],
                                            rhs=xt[:, kd, :],
                                            start=(kd == 0),
                                            stop=(kd == KD - 1),
                                        )
                                for nt in range(NT):
                                    dst = (
                                        qT_b[blk][:, nt, :]
                                        if kind == "q0"
                                        else kT[:, nt,
                                                blk * SBLK : (blk + 1) * SBLK]
                                    )
                                    nc.vector.tensor_scalar_add(
                                        dst, psp[nt], b_sb[:, nt : nt + 1]
                                    )

                # xq blocks 1..3 staged for the phase-B q-projection drip
                xq_drip = []
                for blk in range(1, NB):
                    xt = pa.tile([P, KD, SBLK], BF16, tag="xqd")
                    nc.sync.dma_start(xt, xblk(xq_d, blk))
                    xq_drip.append((blk, xt))

                # ---- Phase B: global chunk stream over (ib, hp, jt) ----
                with tc.tile_pool(name="phb", bufs=2) as pb, \
                     tc.tile_pool(name="psb", bufs=1, space="PSUM") as psb:

                    state = {"chunk": 0}
                    drip = []      # (ready_chunk, emit_fn)
                    ps_o_of = {}   # pair -> [ps_o0, ps_o1]

                    def get_ps_o(pair):
                        if pair not in ps_o_of:
                            ps_o_of[pair] = [
                                psb.tile([P, SBLK], F32, tag=f"ps_o{h01}",
                                         bufs=1, name=f"ps_o{h01}")
                                for h01 in range(2)
                            ]
                        return ps_o_of[pair]

                    def emit_exp_pv(pair, jt, ps_s):
                        ib, hp = pair
                        ps_o = get_ps_o(pair)
                        p_sb = pb.tile([P, 2, SBLK], BF16, tag="p_sb",
                                       bufs=4, name="p_sb")
                        nc.scalar.activation(
                            p_sb, ps_s,
                            mybir.ActivationFunctionType.Exp,
                            scale=float(1.0 / np.sqrt(DH)),
                        )
                        for h01 in range(2):
                            h = hp * 2 + h01
                            nc.tensor.matmul(
                                ps_o[h01][0 : DH + 1, :],
                                lhsT=v_sb[:, jt, h, :],
                                rhs=p_sb[:, h01, :],
                                start=(jt == 0),
                                stop=(jt == JT - 1),
                            )
                        if jt == JT - 1:
                            emit_normalize(pair)

                    def emit_normalize(pair):
                        ib, hp = pair
                        ps_o = ps_o_of[pair]
                        last = ib == NB - 1 and hp == NT - 1
                        if not last:
                            o_un = pb.tile([DH + 1, 2, SBLK], F32,
                                           tag="o_un", bufs=2)
                            for h01 in range(2):
                                nc.vector.tensor_copy(
                                    o_un[:, h01, :], ps_o[h01][0 : DH + 1, :]
                                )
                            den0 = pb.tile([1, 2, SBLK], F32, tag="den0",
                                           bufs=2)
                            nc.vector.tensor_copy(
                                den0, o_un[DH : DH + 1, :, :]
                            )
                            rec = pb.tile([1, 2, SBLK], F32, tag="rec",
                                          bufs=2)
                            nc.vector.reciprocal_approx_fast(rec, den0)
                            bc = pb.tile([DH, 2, SBLK], F32, tag="bc",
                                         bufs=2)
                            nc.gpsimd.partition_broadcast(bc, rec)
                            for h01 in range(2):
                                base = h01 * DH
                                o_slice = o_b[ib][base : base + DH, hp, :]
                                nc.vector.tensor_mul(
                                    o_slice, o_un[0:DH, h01, :],
                                    bc[:, h01, :]
                                )
                        else:
                            # tail pair: per-head chains straight from PSUM
                            # (shortest serial latency; no WAR pressure after
                            # this point).
                            for h01 in range(2):
                                base = h01 * DH
                                dn = pb.tile([1, SBLK], F32,
                                             tag=f"dn{h01}", bufs=1,
                                             name=f"dn{h01}")
                                nc.vector.tensor_copy(
                                    dn, ps_o[h01][DH : DH + 1, :]
                                )
                                rc = pb.tile([1, SBLK], F32,
                                             tag=f"rc{h01}", bufs=1,
                                             name=f"rc{h01}")
                                nc.vector.reciprocal_approx_fast(rc, dn)
                                bch = pb.tile([DH, SBLK], F32,
                                              tag=f"bch{h01}", bufs=1,
                                              name=f"bch{h01}")
                                nc.gpsimd.partition_broadcast(bch, rc)
                                o_slice = o_b[ib][base : base + DH, hp, :]
                                nc.vector.tensor_mul(
                                    o_slice, ps_o[h01][0:DH, :], bch
                                )
                        del ps_o_of[pair]
                        c = state["chunk"]
                        if ib < NB - 1:
                            if hp == NT - 1:
                                for j, st in enumerate(
                                        range(ib * SS, (ib + 1) * SS)):
                                    drip.append((
                                        c + 4 + 2 * j,
                                        make_outproj(ib, st, 0, NT,
                                                     out_d, None, False),
                                    ))
                        else:
                            for j, st in enumerate(
                                    range(ib * SS, (ib + 1) * SS)):
                                if hp == 0:
                                    drip.append((
                                        c + 2 + 3 * j,
                                        make_outproj(ib, st, 0, 1,
                                                     out_d, None, False),
                                    ))
                                else:
                                    drip.append((
                                        c,
                                        make_outproj(ib, st, 1, NT,
                                                     out1_d, (ib * SS) * P,
                                                     True),
                                    ))

                    def make_outproj(ib, st, t0, t1, dst_d, row_off, tail):
                        # full-D output chunk: 1024-wide moving dim uses the
                        # whole 2-bank rotation tile in one go (half the
                        # instruction / copy / DMA count of 512-wide chunks).
                        def emit():
                            pso = psb.tile([P, 2, SBLK], F32, tag="ps_s",
                                           bufs=3, name="pso")
                            for t in range(t0, t1):
                                ss_off = (st - ib * SS) * P
                                for a in range(2):
                                    nc.tensor.matmul(
                                        pso[:, a, :],
                                        lhsT=o_b[ib][:, t,
                                                     ss_off : ss_off + P],
                                        rhs=wo_sb[:, t,
                                                  a * SBLK : (a + 1) * SBLK],
                                        start=(t == t0),
                                        stop=(t == t1 - 1),
                                    )
                            ob = pb.tile([P, 2, SBLK], BF16, tag="ob",
                                         bufs=3)
                            if tail:
                                # ScalarE is idle after the last exp — split
                                # the evacuation across both engines.
                                nc.scalar.copy(ob[:, 0, :], pso[:, 0, :])
                                nc.vector.tensor_copy(ob[:, 1, :],
                                                      pso[:, 1, :])
                            else:
                                nc.vector.tensor_copy(ob, pso)
                            r0 = st * P - (row_off or 0)
                            nc.sync.dma_start(
                                dst_d[r0 : r0 + P, :],
                                ob.rearrange("p a s -> p (a s)"),
                            )
                        return emit

                    def make_qproj(blk, xt, nt):
                        def emit():
                            psp = psb.tile([P, 2, SBLK], F32, tag="ps_s",
                                           bufs=3, name="qp")
                            for kd in range(KD):
                                nc.tensor.matmul(
                                    psp[:, nt, :],
                                    lhsT=wq_sb[:, kd,
                                               nt * P : (nt + 1) * P],
                                    rhs=xt[:, kd, :],
                                    start=(kd == 0),
                                    stop=(kd == KD - 1),
                                )
                            nc.vector.tensor_scalar_add(
                                qT_b[blk][:, nt, :], psp[:, nt, :],
                                bq_sb[:, nt : nt + 1],
                            )
                        return emit

                    for i, (blk, xt) in enumerate(xq_drip):
                        for nt in range(NT):
                            drip.append((
                                4 + 16 * i + 6 * nt,
                                make_qproj(blk, xt, nt),
                            ))

                    pipe = []
                    for ib in range(NB):
                        for hp in range(NT):
                            pair = (ib, hp)
                            for jt in range(JT):
                                ps_s = psb.tile([P, 2, SBLK], F32,
                                                tag="ps_s", bufs=3,
                                                name="ps_s")
                                for h01 in range(2):
                                    base = h01 * DH
                                    nc.tensor.matmul(
                                        ps_s[:, h01, :],
                                        lhsT=kT[base : base + DH, hp,
                                                jt * P : (jt + 1) * P],
                                        rhs=qT_b[ib][base : base + DH,
                                                     hp, :],
                                        start=True,
                                        stop=True,
                                        tile_position=(base, 0),
                                    )
                                state["chunk"] += 1
                                if len(pipe) == 2:
                                    emit_exp_pv(*pipe.pop(0))
                                if drip and drip[0][0] <= state["chunk"]:
                                    drip.pop(0)[1]()
                                pipe.append((pair, jt, ps_s))
                    while pipe:
                        emit_exp_pv(*pipe.pop(0))
                    while drip:
                        drip.pop(0)[1]()

    nc.finalize()
    return nc


_NC_CACHE = {}


def _get_program(S, D, NH):
    key = (S, D, NH)
    if key not in _NC_CACHE:
        _NC_CACHE[key] = build_core_program(S, D, NH)
    return _NC_CACHE[key]


def _pack_x(x, bf16, P, KD, SBLK, NB):
    """[S, D] fp32 -> [NB*P, KD*SBLK] bf16, partition-major per block:
    out[blk*P + p, kd*SBLK + s] = x[blk*SBLK + s, kd*P + p]."""
    y = x.astype(bf16)
    z = y.reshape(NB, SBLK, KD, P).transpose(0, 3, 2, 1)
    return np.ascontiguousarray(z).reshape(NB * P, KD * SBLK)


def make_in_maps(q, k, v, Wq, bq, Wk, bk, Wv, bv, Wo):
    """Host-side sharding + packing into the device's partition-major
    layouts. Returns the per-core input maps."""
    import ml_dtypes

    bf16 = ml_dtypes.bfloat16
    B, S, D = q.shape
    GROUPS = N_CORES // B
    NSL = D // GROUPS
    P = 128
    KD = D // P
    NT = NSL // P
    SBLK = 512 if S % 512 == 0 else S
    NB = S // SBLK

    q, k, v = (np.asarray(x, np.float32) for x in (q, k, v))
    xqt = [_pack_x(q[b], bf16, P, KD, SBLK, NB) for b in range(B)]
    xkt = [_pack_x(k[b], bf16, P, KD, SBLK, NB) for b in range(B)]
    xvt = [_pack_x(v[b], bf16, P, KD, SBLK, NB) for b in range(B)]
    Wq, Wk, Wv, Wo = (np.asarray(x, np.float32) for x in (Wq, Wk, Wv, Wo))
    bq, bk = (np.asarray(x, np.float32) for x in (bq, bk))

    def pack_w(w):  # [D, NSL] -> [P, KD*NSL]
        return np.ascontiguousarray(
            w.astype(bf16).reshape(KD, P, NSL).transpose(1, 0, 2)
        ).reshape(P, KD * NSL)

    def pack_wo(w):  # [NSL, D] -> [P, NT*D]
        return np.ascontiguousarray(
            w.astype(bf16).reshape(NT, P, D).transpose(1, 0, 2)
        ).reshape(P, NT * D)

    def pack_b(b):  # [NSL] -> [P, NT]
        return np.ascontiguousarray(b.reshape(NT, P).T)

    in_maps = []
    for c in range(N_CORES):
        b, hg = c // GROUPS, c % GROUPS
        sl = slice(hg * NSL, (hg + 1) * NSL)
        in_maps.append(
            {
                "xqt": xqt[b],
                "xkt": xkt[b],
                "xvt": xvt[b],
                "wq": pack_w(Wq[:, sl]),
                "wk": pack_w(Wk[:, sl]),
                "wv": pack_w(Wv[:, sl]),
                "wo": pack_wo(np.ascontiguousarray(Wo[sl, :])),
                "bq": pack_b(bq[sl]),
                "bk": pack_b(bk[sl]),
            }
        )
    return in_maps


def kernel(q, k, v, Wq, bq, Wk, bk, Wv, bv, Wo, bo):
    B, S, D = q.shape
    GROUPS = N_CORES // B
    NSL = D // GROUPS
    SBLK = 512 if S % 512 == 0 else S

    nc = _get_program(S, D, NSL // DH)
    in_maps = make_in_maps(q, k, v, Wq, bq, Wk, bk, Wv, bv, Wo)
    res = run_bass_kernel_spmd(nc, in_maps, list(range(N_CORES)))

    out = np.zeros((B, S, D), np.float32)
    for c in range(N_CORES):
        b = c // GROUPS
        out[b] += np.asarray(res.results[c]["out"], np.float32)
        out[b, S - SBLK :] += np.asarray(res.results[c]["out1"], np.float32)
    # bv commutes through the softmax (rows sum to 1): P @ (V + bv) @ Wo =
    # P @ V @ Wo + bv @ Wo. Apply bv@Wo + bo host-side.
    bias = np.asarray(bv, np.float32) @ np.asarray(Wo, np.float32)
    bias += np.asarray(bo, np.float32)
    out += bias[None, None, :]
    return out


# revision 19
# speedup vs baseline: 1.0103x; 1.0103x over previous
"""Multi-head attention (B=2, S=2048, D=1024, H=16) on 8 Trainium2 cores.

Sharding: core c = (batch b, head-group hg) with b = c // 4, hg = c % 4.
Each core computes 4 heads of one batch element end-to-end and emits a
partial output projection; the host sums the 4 partials per batch and adds
bo + bv @ Wo (the V-bias commutes through softmax-normalized attention).

Performance-critical structure (v6):
  - ALL device inputs are pre-packed on the host into partition-major
    layouts (x as [NB*128, KD*SBLK] bf16 blocks, weights as [128, kd*n])
    so every DMA is one large contiguous descriptor per partition — the
    descriptor-fragmented rearranges cost a 16 us DMA head and ~30% of
    aggregate bandwidth in earlier revisions.
  - All matmul operands bf16; PSUM fp32. Projection accumulation chains
    interleave across PSUM banks (4-way for the 256-free V chains) so the
    accumulate turnaround never serializes the PE.
  - Phase B is paced by ScalarE exp (16.8M exps/core, ~1.14 us per
    [128,1024] tile = the hard floor). Score chunks are per-j-tile with both
    heads packed in one 2-bank PSUM tile (3 rotate; PV accumulators pin 2),
    exp->PV runs 2 chunks deep through a global chunk stream that crosses
    head-pair/block boundaries, and softmax normalize runs entirely on
    DVE+GPSIMD (fast reciprocal + partition broadcast).
  - q-projections for blocks 1..3 (split per head-pair) and finished
    blocks' out-projections drip into the chunk stream as PE filler.
  - Tail: the last block's out-projection splits by head-pair (t=0 early
    into `out`, t=1 into `out1`, summed host-side) and its normalize runs
    per-head directly from PSUM to shorten the critical chain.
"""

import numpy as np

import concourse.bacc as bacc
import concourse.mybir as mybir
import concourse.tile as tile
from concourse.bass_utils import run_bass_kernel_spmd

F32 = mybir.dt.float32
BF16 = mybir.dt.bfloat16

S_FULL, D_FULL, NH_PER_CORE, DH = 2048, 1024, 4, 64
N_CORES = 8
B_FULL, H_FULL = 2, 16


def build_core_program(S=S_FULL, D=D_FULL, NH=NH_PER_CORE):
    """One core's program. Packed inputs (bf16 unless noted):
      xqt/xkt/xvt [NB*P, KD*SBLK]  x^T blocks, partition-major
      wq/wk/wv    [P, KD*NSL]      projection weights, partition-major
      wo          [P, NT*D]        out-proj weights, partition-major
      bq/bk       [P, NT] f32
    Outputs: out [S,D] (partial; last block rows hold only the t=0 half)
    and out1 [SBLK,D] (last block t=1 half)."""
    NSL = NH * DH
    P = 128
    KD = D // P
    NT = NSL // P
    SBLK = 512 if S % 512 == 0 else S
    NB = S // SBLK
    JT = S // P
    SS = SBLK // P

    nc = bacc.Bacc("TRN2", target_bir_lowering=False, debug=False)

    xq_d = nc.dram_tensor("xqt", [NB * P, KD * SBLK], BF16,
                          kind="ExternalInput")
    xk_d = nc.dram_tensor("xkt", [NB * P, KD * SBLK], BF16,
                          kind="ExternalInput")
    xv_d = nc.dram_tensor("xvt", [NB * P, KD * SBLK], BF16,
                          kind="ExternalInput")
    wq_d = nc.dram_tensor("wq", [P, KD * NSL], BF16, kind="ExternalInput")
    wk_d = nc.dram_tensor("wk", [P, KD * NSL], BF16, kind="ExternalInput")
    wv_d = nc.dram_tensor("wv", [P, KD * NSL], BF16, kind="ExternalInput")
    wo_d = nc.dram_tensor("wo", [P, NT * D], BF16, kind="ExternalInput")
    bq_d = nc.dram_tensor("bq", [P, NT], F32, kind="ExternalInput")
    bk_d = nc.dram_tensor("bk", [P, NT], F32, kind="ExternalInput")
    out_d = nc.dram_tensor("out", [S, D], BF16, kind="ExternalOutput")
    out1_d = nc.dram_tensor("out1", [SBLK, D], BF16, kind="ExternalOutput")

    def xblk(x_d, blk):
        return x_d[blk * P : (blk + 1) * P, :].rearrange(
            "p (kd s) -> p kd s", kd=KD
        )

    with tile.TileContext(nc) as tc:
        with tc.tile_pool(name="persist", bufs=1) as pp:
            wv_sb = pp.tile([P, KD, NSL], BF16)
            nc.sync.dma_start(
                wv_sb, wv_d.rearrange("p (kd n) -> p kd n", kd=KD)
            )
            wk_sb = pp.tile([P, KD, NSL], BF16)
            wq_sb = pp.tile([P, KD, NSL], BF16)
            wo_sb = pp.tile([P, NT, D], BF16)
            bq_sb = pp.tile([P, NT], F32)
            bk_sb = pp.tile([P, NT], F32)

            qT_b = [
                pp.tile([P, NT, SBLK], BF16, name=f"qT{b}") for b in range(NB)
            ]
            kT = pp.tile([P, NT, S], BF16)
            v_sb = pp.tile([P, JT, NH, DH + 1], BF16)
            nc.vector.memset(v_sb[:, :, :, DH : DH + 1], 1.0)
            o_b = [
                pp.tile([P, NT, SBLK], BF16, name=f"o{b}") for b in range(NB)
            ]

            with tc.tile_pool(name="stage", bufs=3) as pa:
                # ---- Phase A: v + k projections for all blocks, q block 0.
                with tc.tile_pool(name="psa", bufs=2, space="PSUM") as psa:
                    # Pre-ramp: the PE DVFS reaches its top p-state only
                    # after a few microseconds of continuous execution, and
                    # the input DMA head would otherwise leave it cold for
                    # the first half of the v-projection. Chew ~10 us of
                    # dependency-free dummy matmuls while the DMA lands.
                    dum = pp.tile([P, 128], BF16)
                    nc.vector.memset(dum, 0.0)
                    psd = [
                        psa.tile([P, 128], F32, tag="psv", bufs=4,
                                 name=f"psd{i}")
                        for i in range(2)
                    ]
                    for i in range(24):
                        nc.tensor.matmul(
                            psd[i % 2],
                            lhsT=dum,
                            rhs=dum,
                            start=True,
                            stop=True,
                        )
                    for kind, x_d in (("v", xv_d), ("k", xk_d), ("q0", xq_d)):
                        blks = range(NB) if kind != "q0" else range(1)
                        for blk in blks:
                            xt = pa.tile([P, KD, SBLK], BF16, tag="xt")
                            nc.sync.dma_start(xt, xblk(x_d, blk))
                            if kind == "v" and blk == 0:
                                nc.sync.dma_start(
                                    wk_sb,
                                    wk_d.rearrange("p (kd n) -> p kd n",
                                                   kd=KD),
                                )
                                nc.sync.dma_start(
                                    wq_sb,
                                    wq_d.rearrange("p (kd n) -> p kd n",
                                                   kd=KD),
                                )
                                nc.sync.dma_start(
                                    wo_sb,
                                    wo_d.rearrange("p (t n) -> p t n", t=NT),
                                )
                                nc.sync.dma_start(bq_sb, bq_d[:, :])
                                nc.sync.dma_start(bk_sb, bk_d[:, :])
                            if kind == "v":
                                # 4-way bank interleave: at full clock a
                                # 256-cycle matmul is shorter than the
                                # accumulate turnaround, so 2-way is not
                                # enough to keep the chains pipelined.
                                psv = [
                                    psa.tile([P, NSL], F32, tag="psv",
                                             bufs=4, name=f"psv{i}")
                                    for i in range(SS)
                                ]
                                for kd in range(KD):
                                    for ss in range(SS):
                                        nc.tensor.matmul(
                                            psv[ss],
                                            lhsT=xt[:, kd,
                                                    ss * P : (ss + 1) * P],
                                            rhs=wv_sb[:, kd, :],
                                            start=(kd == 0),
                                            stop=(kd == KD - 1),
                                        )
                                for ss in range(SS):
                                    st = blk * SS + ss
                                    nc.vector.tensor_copy(
                                        v_sb[:, st, :, 0:DH],
                                        psv[ss].rearrange(
                                            "p (h d) -> p h d", d=DH
                                        ),
                                    )
                            else:
                                w_sb = wk_sb if kind == "k" else wq_sb
                                b_sb = bk_sb if kind == "k" else bq_sb
                                psp = [
                                    psa.tile([P, SBLK], F32, tag="psp",
                                             bufs=4, name=f"psp{i}")
                                    for i in range(NT)
                                ]
                                for kd in range(KD):
                                    for nt in range(NT):
                                        nc.tensor.matmul(
                                            psp[nt],
                                            lhsT=w_sb[:, kd,
                                                      nt * P : (nt + 1) * P],
                                            rhs=xt[:, kd, :],
                                            start=(kd == 0),
                                            stop=(kd == KD - 1),
                                        )
                                for nt in range(NT):
                                    dst = (
                                        qT_b[blk][:, nt, :]
                                        if kind == "q0"
                                        else kT[:, nt,
                                                blk * SBLK : (blk + 1) * SBLK]
                                    )
                                    nc.vector.tensor_scalar_add(
                                        dst, psp[nt], b_sb[:, nt : nt + 1]
                                    )

                # xq blocks 1..3 staged for the phase-B q-projection drip
                xq_drip = []
                for blk in range(1, NB):
                    xt = pa.tile([P, KD, SBLK], BF16, tag="xqd")
                    nc.sync.dma_start(xt, xblk(xq_d, blk))
                    xq_drip.append((blk, xt))

                # ---- Phase B: global chunk stream over (ib, hp, jt) ----
                with tc.tile_pool(name="phb", bufs=2) as pb, \
                     tc.tile_pool(name="psb", bufs=1, space="PSUM") as psb:

                    state = {"chunk": 0}
                    drip = []      # (ready_chunk, emit_fn)
                    ps_o_of = {}   # pair -> [ps_o0, ps_o1]

                    def get_ps_o(pair):
                        if pair not in ps_o_of:
                            ps_o_of[pair] = [
                                psb.tile([P, SBLK], F32, tag=f"ps_o{h01}",
                                         bufs=1, name=f"ps_o{h01}")
                                for h01 in range(2)
                            ]
                        return ps_o_of[pair]

                    def emit_exp_pv(pair, jt, ps_s):
                        ib, hp = pair
                        ps_o = get_ps_o(pair)
                        p_sb = pb.tile([P, 2, SBLK], BF16, tag="p_sb",
                                       bufs=4, name="p_sb")
                        nc.scalar.activation(
                            p_sb, ps_s,
                            mybir.ActivationFunctionType.Exp,
                            scale=float(1.0 / np.sqrt(DH)),
                        )
                        for h01 in range(2):
                            h = hp * 2 + h01
                            nc.tensor.matmul(
                                ps_o[h01][0 : DH + 1, :],
                                lhsT=v_sb[:, jt, h, :],
                                rhs=p_sb[:, h01, :],
                                start=(jt == 0),
                                stop=(jt == JT - 1),
                            )
                        if jt == JT - 1:
                            emit_normalize(pair)

                    def emit_normalize(pair):
                        ib, hp = pair
                        ps_o = ps_o_of[pair]
                        last = ib == NB - 1 and hp == NT - 1
                        if not last:
                            # evacuate on ScalarE: at a pair boundary the
                            # exp stream has a natural 1-2 chunk idle window
                            # (pipe refill), and DVE may be busy with drip
                            # copies — ACT clears the ps_o WAR sooner.
                            o_un = pb.tile([DH + 1, 2, SBLK], F32,
                                           tag="o_un", bufs=2)
                            for h01 in range(2):
                                nc.scalar.copy(
                                    o_un[:, h01, :], ps_o[h01][0 : DH + 1, :]
                                )
                            den0 = pb.tile([1, 2, SBLK], F32, tag="den0",
                                           bufs=2)
                            nc.vector.tensor_copy(
                                den0, o_un[DH : DH + 1, :, :]
                            )
                            rec = pb.tile([1, 2, SBLK], F32, tag="rec",
                                          bufs=2)
                            nc.vector.reciprocal_approx_fast(rec, den0)
                            bc = pb.tile([DH, 2, SBLK], F32, tag="bc",
                                         bufs=2)
                            nc.gpsimd.partition_broadcast(bc, rec)
                            for h01 in range(2):
                                base = h01 * DH
                                o_slice = o_b[ib][base : base + DH, hp, :]
                                nc.vector.tensor_mul(
                                    o_slice, o_un[0:DH, h01, :],
                                    bc[:, h01, :]
                                )
                        else:
                            # tail pair: per-head chains straight from PSUM
                            # (shortest serial latency; no WAR pressure after
                            # this point).
                            for h01 in range(2):
                                base = h01 * DH
                                dn = pb.tile([1, SBLK], F32,
                                             tag=f"dn{h01}", bufs=1,
                                             name=f"dn{h01}")
                                nc.vector.tensor_copy(
                                    dn, ps_o[h01][DH : DH + 1, :]
                                )
                                rc = pb.tile([1, SBLK], F32,
                                             tag=f"rc{h01}", bufs=1,
                                             name=f"rc{h01}")
                                nc.vector.reciprocal_approx_fast(rc, dn)
                                bch = pb.tile([DH, SBLK], F32,
                                              tag=f"bch{h01}", bufs=1,
                                              name=f"bch{h01}")
                                nc.gpsimd.partition_broadcast(bch, rc)
                                o_slice = o_b[ib][base : base + DH, hp, :]
                                nc.vector.tensor_mul(
                                    o_slice, ps_o[h01][0:DH, :], bch
                                )
                        del ps_o_of[pair]
                        c = state["chunk"]
                        if ib < NB - 1:
                            if hp == NT - 1:
                                for j, st in enumerate(
                                        range(ib * SS, (ib + 1) * SS)):
                                    drip.append((
                                        c + 4 + 8 * j,
                                        make_outproj(ib, st, 0, NT,
                                                     out_d, None, False),
                                    ))
                        else:
                            for j, st in enumerate(
                                    range(ib * SS, (ib + 1) * SS)):
                                if hp == 0:
                                    drip.append((
                                        c + 2 + 3 * j,
                                        make_outproj(ib, st, 0, 1,
                                                     out_d, None, False),
                                    ))
                                else:
                                    drip.append((
                                        c,
                                        make_outproj(ib, st, 1, NT,
                                                     out1_d, (ib * SS) * P,
                                                     True),
                                    ))

                    def make_outproj(ib, st, t0, t1, dst_d, row_off, tail):
                        # full-D output chunk: 1024-wide moving dim uses the
                        # whole 2-bank rotation tile in one go (half the
                        # instruction / copy / DMA count of 512-wide chunks).
                        def emit():
                            pso = psb.tile([P, 2, SBLK], F32, tag="ps_s",
                                           bufs=3, name="pso")
                            for t in range(t0, t1):
                                ss_off = (st - ib * SS) * P
                                for a in range(2):
                                    nc.tensor.matmul(
                                        pso[:, a, :],
                                        lhsT=o_b[ib][:, t,
                                                     ss_off : ss_off + P],
                                        rhs=wo_sb[:, t,
                                                  a * SBLK : (a + 1) * SBLK],
                                        start=(t == t0),
                                        stop=(t == t1 - 1),
                                    )
                            ob = pb.tile([P, 2, SBLK], BF16, tag="ob",
                                         bufs=3)
                            if tail:
                                # ScalarE is idle after the last exp — split
                                # the evacuation across both engines.
                                nc.scalar.copy(ob[:, 0, :], pso[:, 0, :])
                                nc.vector.tensor_copy(ob[:, 1, :],
                                                      pso[:, 1, :])
                            else:
                                nc.vector.tensor_copy(ob, pso)
                            r0 = st * P - (row_off or 0)
                            nc.sync.dma_start(
                                dst_d[r0 : r0 + P, :],
                                ob.rearrange("p a s -> p (a s)"),
                            )
                        return emit

                    def make_qproj(blk, xt, nt):
                        def emit():
                            psp = psb.tile([P, 2, SBLK], F32, tag="ps_s",
                                           bufs=3, name="qp")
                            for kd in range(KD):
                                nc.tensor.matmul(
                                    psp[:, nt, :],
                                    lhsT=wq_sb[:, kd,
                                               nt * P : (nt + 1) * P],
                                    rhs=xt[:, kd, :],
                                    start=(kd == 0),
                                    stop=(kd == KD - 1),
                                )
                            nc.vector.tensor_scalar_add(
                                qT_b[blk][:, nt, :], psp[:, nt, :],
                                bq_sb[:, nt : nt + 1],
                            )
                        return emit

                    for i, (blk, xt) in enumerate(xq_drip):
                        for nt in range(NT):
                            drip.append((
                                4 + 16 * i + 6 * nt,
                                make_qproj(blk, xt, nt),
                            ))

                    pipe = []
                    for ib in range(NB):
                        for hp in range(NT):
                            pair = (ib, hp)
                            for jt in range(JT):
                                ps_s = psb.tile([P, 2, SBLK], F32,
                                                tag="ps_s", bufs=3,
                                                name="ps_s")
                                for h01 in range(2):
                                    base = h01 * DH
                                    nc.tensor.matmul(
                                        ps_s[:, h01, :],
                                        lhsT=kT[base : base + DH, hp,
                                                jt * P : (jt + 1) * P],
                                        rhs=qT_b[ib][base : base + DH,
                                                     hp, :],
                                        start=True,
                                        stop=True,
                                        tile_position=(base, 0),
                                    )
                                state["chunk"] += 1
                                if len(pipe) == 2:
                                    emit_exp_pv(*pipe.pop(0))
                                if drip and drip[0][0] <= state["chunk"]:
                                    drip.pop(0)[1]()
                                pipe.append((pair, jt, ps_s))
                    while pipe:
                        emit_exp_pv(*pipe.pop(0))
                    while drip:
                        drip.pop(0)[1]()

    nc.finalize()
    return nc


_NC_CACHE = {}


def _get_program(S, D, NH):
    key = (S, D, NH)
    if key not in _NC_CACHE:
        _NC_CACHE[key] = build_core_program(S, D, NH)
    return _NC_CACHE[key]


def _pack_x(x, bf16, P, KD, SBLK, NB):
    """[S, D] fp32 -> [NB*P, KD*SBLK] bf16, partition-major per block:
    out[blk*P + p, kd*SBLK + s] = x[blk*SBLK + s, kd*P + p]."""
    y = x.astype(bf16)
    z = y.reshape(NB, SBLK, KD, P).transpose(0, 3, 2, 1)
    return np.ascontiguousarray(z).reshape(NB * P, KD * SBLK)


def make_in_maps(q, k, v, Wq, bq, Wk, bk, Wv, bv, Wo):
    """Host-side sharding + packing into the device's partition-major
    layouts. Returns the per-core input maps."""
    import ml_dtypes

    bf16 = ml_dtypes.bfloat16
    B, S, D = q.shape
    GROUPS = N_CORES // B
    NSL = D // GROUPS
    P = 128
    KD = D // P
    NT = NSL // P
    SBLK = 512 if S % 512 == 0 else S
    NB = S // SBLK

    q, k, v = (np.asarray(x, np.float32) for x in (q, k, v))
    xqt = [_pack_x(q[b], bf16, P, KD, SBLK, NB) for b in range(B)]
    xkt = [_pack_x(k[b], bf16, P, KD, SBLK, NB) for b in range(B)]
    xvt = [_pack_x(v[b], bf16, P, KD, SBLK, NB) for b in range(B)]
    Wq, Wk, Wv, Wo = (np.asarray(x, np.float32) for x in (Wq, Wk, Wv, Wo))
    bq, bk = (np.asarray(x, np.float32) for x in (bq, bk))

    def pack_w(w):  # [D, NSL] -> [P, KD*NSL]
        return np.ascontiguousarray(
            w.astype(bf16).reshape(KD, P, NSL).transpose(1, 0, 2)
        ).reshape(P, KD * NSL)

    def pack_wo(w):  # [NSL, D] -> [P, NT*D]
        return np.ascontiguousarray(
            w.astype(bf16).reshape(NT, P, D).transpose(1, 0, 2)
        ).reshape(P, NT * D)

    def pack_b(b):  # [NSL] -> [P, NT]
        return np.ascontiguousarray(b.reshape(NT, P).T)

    in_maps = []
    for c in range(N_CORES):
        b, hg = c // GROUPS, c % GROUPS
        sl = slice(hg * NSL, (hg + 1) * NSL)
        in_maps.append(
            {
                "xqt": xqt[b],
                "xkt": xkt[b],
                "xvt": xvt[b],
                "wq": pack_w(Wq[:, sl]),
                "wk": pack_w(Wk[:, sl]),
                "wv": pack_w(Wv[:, sl]),
                "wo": pack_wo(np.ascontiguousarray(Wo[sl, :])),
                "bq": pack_b(bq[sl]),
                "bk": pack_b(bk[sl]),
            }
        )
    return in_maps


def kernel(q, k, v, Wq, bq, Wk, bk, Wv, bv, Wo, bo):
    B, S, D = q.shape
    GROUPS = N_CORES // B
    NSL = D // GROUPS
    SBLK = 512 if S % 512 == 0 else S

    nc = _get_program(S, D, NSL // DH)
    in_maps = make_in_maps(q, k, v, Wq, bq, Wk, bk, Wv, bv, Wo)
    res = run_bass_kernel_spmd(nc, in_maps, list(range(N_CORES)))

    out = np.zeros((B, S, D), np.float32)
    for c in range(N_CORES):
        b = c // GROUPS
        out[b] += np.asarray(res.results[c]["out"], np.float32)
        out[b, S - SBLK :] += np.asarray(res.results[c]["out1"], np.float32)
    # bv commutes through the softmax (rows sum to 1): P @ (V + bv) @ Wo =
    # P @ V @ Wo + bv @ Wo. Apply bv@Wo + bo host-side.
    bias = np.asarray(bv, np.float32) @ np.asarray(Wo, np.float32)
    bias += np.asarray(bo, np.float32)
    out += bias[None, None, :]
    return out
